# revision 5
# baseline (speedup 1.0000x reference)
"""MoE hard-routing kernel for Trainium2 (8 NeuronCores, Bass/Tile).

Problem: out[t] = x[t] @ W[p[t]].T + b[p[t]]
  x [8, 4096, 512] f32, partitions [8, 4096] int32 (values 0..7),
  W [8, 512, 512] f32, b [8, 512] f32.

Strategy: expert-parallel sharding. n_experts == n_cores == 8, so core e
owns expert e. The host routes each token to its expert's core (that IS the
shard assignment — a partition of the token set), pre-transposed so d_in
lies on SBUF partitions, and pre-cast to fp16 (tolerance is 2e-2; fp16
operands keep the GEMM rel-err ~1e-3 while halving DMA traffic and keeping
the PE at its full 1 row/cycle rate). Each core runs one dense GEMM
  out_e[d_out, tok] = W[e] @ xT_e  (+ b[e])
accumulated over 4 K-chunks of 128 in fp32 PSUM, with the bias added and
the fp32->fp16 downcast fused into the PSUM eviction.

Per-trace tuning (HWDGE trigger costs a fixed ~630ns on the issuing queue
and every DMA burns a semaphore that the teardown epilogue later zeroes
serially at ~115ns each):
  - x is loaded one DMA per 1024-column superblock ([128, KC, 1024] tile,
    512 descriptors) instead of per-K-chunk; stores are likewise one DMA
    per superblock.  Block 0 stays per-K so the first matmul only waits on
    a 128KB transfer.
  - W rides the ACT ring as 2 DMAs (k=0 alone so the k=0 matmuls can start
    ~1us earlier); bias rides the otherwise-idle DVE ring.
  - The ACT activation table is preloaded via a dummy Identity activation
    at kernel start so the first real eviction doesn't eat the lazy 1.3us
    ACT_TABLE_LOAD.
  - PSUM evictions alternate DVE (even m) / ACT (odd m) in every block.
  - The final 128-column block keeps the tail short; its store goes on the
    then-idle SP ring.
"""

import sys

for _p in ("/opt/trn_rl_repo", "/root/.axon_site/_ro/trn_rl_repo"):
    if _p not in sys.path:
        sys.path.append(_p)

import numpy as np

import concourse.bass as bass
import concourse.mybir as mybir
import concourse.tile as tile
from concourse.bass import ts
from concourse.bass_utils import run_bass_kernel_spmd
import bass_rust as _br

D_IN = 512
D_OUT = 512
N_EXPERTS = 8
N_CORES = 8
P = 128
NBLK = 512  # token columns per matmul (one PSUM bank of fp32)
SBLK = 1024  # token columns per x-load superblock
KC = D_IN // P  # 4 contraction chunks
MC = D_OUT // P  # 4 output-row chunks

MATH_MODE = "f16"

N_WARMUP = 16  # 256-col warm-up matmuls to ramp the PE clock during DMA
WARM_COLS = 256


def _np_dt(math_mode: str):
    if math_mode == "f16":
        return np.float16
    if math_mode == "bf16":
        import ml_dtypes

        return ml_dtypes.bfloat16
    if math_mode in ("f32r", "f32"):
        return np.float32
    raise ValueError(math_mode)


def _mm_dt(math_mode: str):
    return {
        "f16": mybir.dt.float16,
        "bf16": mybir.dt.bfloat16,
        "f32r": mybir.dt.float32r,
        "f32": mybir.dt.float32,
    }[math_mode]


def _split_multiwait(nc: bass.Bass) -> None:
    """Hoist extra sem waits onto injected same-engine nops.

    The walrus build in this container rejects more than one sync-wait
    command on a single instruction.  Engine queues are in-order, so a
    nop carrying one wait immediately before the instruction is
    semantically identical to the wait being attached directly.
    """
    cnt = 0
    for bb in nc.main_func.blocks:
        new = []
        changed = False
        for ins in bb.instructions:
            si = ins.sync_info
            if si is not None and len(si.on_wait) > 1:
                waits = list(si.on_wait)
                for w in waits[:-1]:
                    nop = mybir.InstNoOp(name=f"wsplit-{cnt}", ins=[], outs=[])
                    cnt += 1
                    nop.engine = ins.engine
                    nop.sync_info = _br.SyncInfo(on_wait=[w], on_update=[])
                    new.append(nop)
                ins.sync_info = _br.SyncInfo(
                    on_wait=[waits[-1]], on_update=list(si.on_update)
                )
                changed = True
            new.append(ins)
        if changed:
            bb.instructions = new


def _build_nc(C: int, math_mode: str) -> bass.Bass:
    """One core's program: out[512, C] = wT.T-contract(xT) + bias."""
    f32 = mybir.dt.float32
    nc = bass.Bass("TRN2", target_bir_lowering=False, debug=False, num_devices=N_CORES)

    mm_dt = _mm_dt(math_mode)
    out_dt = f32 if math_mode in ("f32r", "f32") else mm_dt

    xT = nc.declare_dram_parameter("xT", [D_IN, C], mm_dt, isOutput=False)
    wT = nc.declare_dram_parameter("wT", [D_IN, D_OUT], mm_dt, isOutput=False)
    bias = nc.declare_dram_parameter("bias", [D_OUT], f32, isOutput=False)
    out = nc.declare_dram_parameter("out", [D_OUT, C], out_dt, isOutput=True)

    # [p, k, c] views of the DRAM operands (k-chunk on the free dims).
    xT_v = xT.rearrange("(k p) c -> p k c", p=P)
    out_v = out.rearrange("(m p) c -> p m c", p=P)
    wT_v = wT.rearrange("(k p) d -> p k d", p=P)

    # Superblocks: a leading 512 block (loaded per-K so the first matmul
    # waits on only 128KB), then 1024 blocks, then a 512 block and/or a
    # short (<512) remainder — the short final block keeps the tail short.
    col_blocks = [(0, min(NBLK, C))]
    off = col_blocks[0][1]
    while off < C:
        rem = C - off
        if rem >= SBLK:
            size = SBLK
        elif rem > NBLK:
            size = NBLK
        else:
            size = rem
        col_blocks.append((off, size))
        off += size

    with tile.TileContext(nc) as tc:
        with (
            tc.tile_pool(name="wpool", bufs=1) as wpool,
            tc.tile_pool(name="xpool", bufs=4) as xpool,
            tc.tile_pool(name="x0pool", bufs=1) as x0pool,
            tc.tile_pool(name="opool", bufs=3) as opool,
            tc.tile_pool(name="pspool", bufs=8, space="PSUM") as pspool,
        ):
            # Weights: wT[d_in, d_out] -> [128, KC, 512]; chunk (k, m) is the
            # stationary operand [K=128, M=128].  k=0 goes alone so the k=0
            # matmul stream can start as soon as 128KB lands.
            w_t = wpool.tile([P, KC, D_OUT], mm_dt)
            nc.scalar.dma_start(w_t[:, 0, :], wT_v[:, 0, :])
            nc.scalar.dma_start(w_t[:, 1:, :], wT_v[:, 1:, :])
            # Warm-up matmuls on a zeroed tile keep the PE busy while the
            # first x/W chunks are in flight so its clock ramps to 2.4 GHz.
            warm_x = wpool.tile([P, WARM_COLS], mybir.dt.bfloat16)
            nc.gpsimd.memset(warm_x[:], 0)
            # bias[d_out] -> [128, MC] on the otherwise-idle SWDGE ring
            # (HWDGE exists only on SP/ACT), behind the warm-up memset.
            b_t = wpool.tile([P, MC], f32)
            nc.gpsimd.dma_start(b_t[:], bias.rearrange("(m p) -> p m", p=P))
            # Preload the ACT activation table (Identity) so the first real
            # eviction doesn't pay the lazy 1.3us table load.
            warm_o = wpool.tile([P, 1], f32)
            nc.scalar.activation(
                warm_o[:],
                warm_x[:, :1],
                mybir.ActivationFunctionType.Identity,
                bias=0.0,
            )
            for wi in range(N_WARMUP):
                ps = pspool.tile([P, NBLK], f32, name=f"ps_w{wi}", tag="ps")
                nc.tensor.matmul(
                    ps[:, :WARM_COLS],
                    warm_x[:, :P],
                    warm_x[:],
                    start=True,
                    stop=True,
                )

            n_blocks = len(col_blocks)
            for n, (coff, csz) in enumerate(col_blocks):
                is_last = n == n_blocks - 1
                if n == 0:
                    # Per-K loads: the first matmul only waits on chunk 0.
                    x_t = x0pool.tile([P, KC, csz], mm_dt, name="x_0", tag="x0")
                    for k in range(KC):
                        nc.sync.dma_start(
                            x_t[:, k, :], xT_v[:, k, coff : coff + csz]
                        )
                else:
                    x_t = xpool.tile([P, KC, SBLK], mm_dt, name=f"x_{n}", tag="x")
                    nc.sync.dma_start(
                        x_t[:, :, :csz], xT_v[:, :, coff : coff + csz]
                    )
                o_t = opool.tile([P, MC, csz], out_dt, name=f"o_{n}", tag="o")
                subs = [(o, min(NBLK, csz - o)) for o in range(0, csz, NBLK)]
                for bi, (boff, bsz) in enumerate(subs):
                    ps_m = [
                        pspool.tile([P, NBLK], f32, name=f"ps_{n}_{bi}_{m}", tag="ps")
                        for m in range(MC)
                    ]
                    for k in range(KC):
                        for m in range(MC):
                            nc.tensor.matmul(
                                ps_m[m][:, :bsz],
                                w_t[:, k, ts(m, P)],
                                x_t[:, k, boff : boff + bsz],
                                start=(k == 0),
                                stop=(k == KC - 1),
                            )
                    for m in range(MC):
                        if m % 2 == 1:
                            nc.scalar.activation(
                                o_t[:, m, boff : boff + bsz],
                                ps_m[m][:, :bsz],
                                mybir.ActivationFunctionType.Identity,
                                bias=b_t[:, m : m + 1],
                            )
                        else:
                            nc.vector.tensor_scalar_add(
                                o_t[:, m, boff : boff + bsz],
                                ps_m[m][:, :bsz],
                                b_t[:, m : m + 1],
                            )
                # One store DMA per superblock; the final (short) block rides
                # the by-then-idle SP ring to shorten the tail.
                store_eng = nc.sync if is_last else nc.scalar
                store_eng.dma_start(
                    out_v[:, :, coff : coff + csz], o_t[:, :, :csz]
                )
    _split_multiwait(nc)
    return nc


_NC_CACHE: dict = {}


def _get_nc(C: int, math_mode: str) -> bass.Bass:
    key = (C, math_mode)
    if key not in _NC_CACHE:
        _NC_CACHE[key] = _build_nc(C, math_mode)
    return _NC_CACHE[key]


def kernel(x: np.ndarray, partitions: np.ndarray, W: np.ndarray, b: np.ndarray,
           _math_mode: str | None = None, _trace: bool = False):
    math_mode = _math_mode or MATH_MODE
    np_dt = _np_dt(math_mode)
    B, S, d_in = x.shape
    n_exp, d_out, _ = W.shape
    assert d_in == D_IN and d_out == D_OUT and n_exp == N_EXPERTS

    xf = np.ascontiguousarray(x, dtype=np.float32).reshape(-1, d_in)
    p = partitions.reshape(-1)

    tok_ids = [np.nonzero(p == e)[0] for e in range(N_EXPERTS)]
    max_cnt = max(len(ids) for ids in tok_ids)
    C = max(NBLK, ((max_cnt + P - 1) // P) * P)

    in_maps = []
    for e in range(N_EXPERTS):
        ids = tok_ids[e]
        xT = np.zeros((D_IN, C), np_dt)
        xT[:, : len(ids)] = xf[ids].T.astype(np_dt)
        in_maps.append(
            {
                "xT": xT,
                "wT": np.ascontiguousarray(W[e].T).astype(np_dt),
                "bias": np.ascontiguousarray(b[e], dtype=np.float32),
            }
        )

    nc = _get_nc(C, math_mode)
    res = run_bass_kernel_spmd(nc, in_maps, list(range(N_CORES)), trace=_trace)

    outf = np.empty((B * S, d_out), np.float32)
    for e in range(N_EXPERTS):
        ids = tok_ids[e]
        outf[ids] = np.asarray(res.results[e]["out"])[:, : len(ids)].T.astype(
            np.float32
        )
    out = outf.reshape(B, S, d_out)
    if _trace:
        return out, res
    return out
